# revision 1
# baseline (speedup 1.0000x reference)
"""Distributed 2-layer GCN (BangaloreGCN) on 8 Trainium2 NeuronCores.

Strategy (node/graph-parallel, per spec sharding hint):
  * Nodes are packed into 8*49 destination tiles of 128 slots (LPT on
    in-degree so every tile's incoming-edge count fits a fixed chunk
    budget -> fully static SPMD program).
  * GCN algebra is refactored so message passing is a pure gather +
    segment-sum:  out = dinv * (A @ (dinv*h)) + dinv^2 * h, with the
    per-channel BN scale folded into W, biases folded into a post-add.
  * Per layer: each core computes its shard of the (dinv*h) table,
    AllGather replicates it to HBM on every core, then each core
    dma_gathers the source rows for its own edges and segment-sums them
    with one-hot selection matmuls into PSUM (128 dests x 64 feats).
  * int16 gather indices only span 32768 rows, so edges are split into
    a "low" pass (table rows [0, 32768)) and "high" pass (rows
    [NSLOT-32768, NSLOT)); edges in the overlap are assigned to balance
    per-tile chunk counts.
"""

import sys

sys.path.insert(0, "/opt/trn_rl_repo")

import heapq

import ml_dtypes
import numpy as np

BF16 = ml_dtypes.bfloat16

# ---- problem constants (hardcoded per contest contract) ----
N_NODES = 50000
IN_CH = 128
HID = 64
HID2 = 32
BN_EPS = 1e-5

NCORES = 8
P = 128
TILES = 49                 # dest tiles per core
SPC = TILES * P            # slots per core (6272)
NSLOT = NCORES * SPC       # 50176
NBINS = NCORES * TILES
LO_LIM = 32768             # low gather table covers rows [0, 32768)
HI_BASE = NSLOT - 32768    # high table covers [HI_BASE, NSLOT)
GT = 7                     # dest tiles per dma_gather call
NCALLS = TILES // GT
PAD_DEST = 200.0
TBW = 128                  # padded table row width (bf16 -> 256B elems)

USE_BF16 = True


# ----------------------------------------------------------------------
# host-side preparation
# ----------------------------------------------------------------------
def _pack_nodes(deg_in, n):
    order = np.argsort(-deg_in, kind="stable")
    heap = [(0, b) for b in range(NBINS)]
    heapq.heapify(heap)
    counts = np.zeros(NBINS, np.int32)
    binof = np.empty(n, np.int32)
    for v in order:
        load, b = heapq.heappop(heap)
        binof[v] = b
        counts[b] += 1
        if counts[b] < P:
            heapq.heappush(heap, (load + int(deg_in[v]), b))
    perm = np.argsort(binof, kind="stable")
    ptr = np.zeros(NBINS, np.int32)
    lanes = np.empty(n, np.int32)
    for v in perm:
        b = binof[v]
        lanes[v] = ptr[b]
        ptr[b] += 1
    return binof.astype(np.int64) * P + lanes


def _wrap_idx(arr):
    ni = arr.shape[0]
    blk = arr.reshape(ni // 16, 16).T.astype(np.int16)
    return np.tile(blk, (8, 1))


def host_prep(x, edge_index, W1, b1, W2, b2, fcW, fcb,
              g1, be1, rm1, rv1, g2, be2, rm2, rv2):
    n = x.shape[0]
    row = np.asarray(edge_index[0], np.int64)
    col = np.asarray(edge_index[1], np.int64)

    deg = np.bincount(col, minlength=n).astype(np.float32) + 1.0
    dinv = (1.0 / np.sqrt(deg)).astype(np.float32)
    deg_in = np.bincount(col, minlength=n)

    slot_of_node = _pack_nodes(deg_in, n)
    node_of_slot = np.full(NSLOT, -1, np.int64)
    node_of_slot[slot_of_node] = np.arange(n)

    src_slot = slot_of_node[row]
    dst_slot = slot_of_node[col]
    dbin = dst_slot // P
    dlane = dst_slot % P

    order = np.argsort(dbin, kind="stable")
    src_s = src_slot[order]
    dlane_s = dlane[order]
    dbin_s = dbin[order]
    starts = np.searchsorted(dbin_s, np.arange(NBINS))
    ends = np.searchsorted(dbin_s, np.arange(NBINS) + 1)

    nA_min = np.zeros(NBINS, np.int64)
    nB_min = np.zeros(NBINS, np.int64)
    tot = ends - starts
    for b in range(NBINS):
        s = src_s[starts[b]:ends[b]]
        nA_min[b] = int((s < HI_BASE).sum())
        nB_min[b] = int((s >= LO_LIM).sum())
    maxA, maxB, maxT = int(nA_min.max()), int(nB_min.max()), int(tot.max())
    best = None
    for ct in range(-(-maxT // P), -(-maxT // P) + 8):
        for ca in range(-(-maxA // P), ct + 1):
            cb = ct - ca
            if cb >= 0 and cb * P >= maxB:
                best = (ca, cb)
                break
        if best:
            break
    CA, CB = best
    capA, capB = CA * P, CB * P

    srcA = np.zeros((NBINS, capA), np.int64)
    destA = np.full((NBINS, capA), PAD_DEST, np.float32)
    srcB = np.zeros((NBINS, capB), np.int64)
    destB = np.full((NBINS, capB), PAD_DEST, np.float32)
    for b in range(NBINS):
        s = src_s[starts[b]:ends[b]]
        d = dlane_s[starts[b]:ends[b]]
        isB_must = s >= LO_LIM
        isA_must = s < HI_BASE
        mid_idx = np.where(~isB_must & ~isA_must)[0]
        room = capB - int(isB_must.sum())
        takeB = mid_idx[:room]
        selB = np.concatenate([np.where(isB_must)[0], takeB])
        selA = np.concatenate([np.where(isA_must)[0], mid_idx[room:]])
        assert len(selB) <= capB and len(selA) <= capA
        srcB[b, :len(selB)] = s[selB] - HI_BASE
        destB[b, :len(selB)] = d[selB]
        srcA[b, :len(selA)] = s[selA]
        destA[b, :len(selA)] = d[selA]

    S1c = (g1 / np.sqrt(rv1 + BN_EPS)).astype(np.float32)
    T1 = ((b1 - rm1) * S1c + be1).astype(np.float32)
    S2c = (g2 / np.sqrt(rv2 + BN_EPS)).astype(np.float32)
    T2 = ((b2 - rm2) * S2c + be2).astype(np.float32)
    W1p = (W1 * S1c[None, :]).astype(np.float32)
    W2p = (W2 * S2c[None, :]).astype(np.float32)

    NCH = CA + CB
    cores = []
    for c in range(NCORES):
        tsl = slice(c * TILES, (c + 1) * TILES)
        sA = srcA[tsl].reshape(-1)
        sB = srcB[tsl].reshape(-1)
        idxA_img = np.hstack(
            [_wrap_idx(sA[g * GT * capA:(g + 1) * GT * capA]) for g in range(NCALLS)])
        idxB_img = np.hstack(
            [_wrap_idx(sB[g * GT * capB:(g + 1) * GT * capB]) for g in range(NCALLS)])
        dst_img = np.zeros((P, TILES * NCH), np.float32)
        for tl in range(TILES):
            b = c * TILES + tl
            dst_img[:, tl * NCH:tl * NCH + CA] = destA[b].reshape(CA, P).T
            dst_img[:, tl * NCH + CA:(tl + 1) * NCH] = destB[b].reshape(CB, P).T
        nodes = node_of_slot[c * SPC:(c + 1) * SPC]
        occ = nodes >= 0
        xs = np.zeros((SPC, IN_CH), np.float32)
        xs[occ] = x[nodes[occ]] * dinv[nodes[occ], None]
        dv = np.zeros(SPC, np.float32)
        dv[occ] = dinv[nodes[occ]]
        cores.append(dict(
            idxA=idxA_img, idxB=idxB_img,
            dest=dst_img.astype(BF16) if USE_BF16 else dst_img,
            xT=np.ascontiguousarray(xs.T),
            dinv=np.ascontiguousarray(dv.reshape(TILES, P).T),
        ))

    consts = dict(W1p=W1p, W2p=W2p, T1=T1, T2=T2,
                  fcW=np.asarray(fcW, np.float32), fcb=float(np.asarray(fcb).reshape(-1)[0]),
                  CA=CA, CB=CB, node_of_slot=node_of_slot)
    return cores, consts


# ----------------------------------------------------------------------
# device program
# ----------------------------------------------------------------------
def _dma_gather_raw(gp, bassmod, out_ap, in_ap, idxs_ap, num_idxs, elem_size,
                    elem_step, single_packet=True, queue_num=0):
    """bass.dma_gather with elem_size_bytes below 256B allowed (stride must
    still be a multiple of 256B). Verified on HW (see work/smoke4.py)."""
    import concourse.mybir as mybir
    from concourse import ap_utils
    from concourse.bass import MemorySpace, exact_div, round_up_to_multiple

    assert idxs_ap.dtype == mybir.dt.int16
    assert in_ap.dtype == out_ap.dtype
    assert in_ap.space == MemorySpace.DRAM
    assert idxs_ap.space == MemorySpace.SBUF and out_ap.space == MemorySpace.SBUF
    assert ap_utils.ap_is_contiguous(out_ap.ap[1:])
    assert ap_utils.ap_is_contiguous(idxs_ap.ap[1:])
    assert in_ap.ap[-1][1] == out_ap.ap[-1][1] == elem_size
    assert out_ap.ap[0][1] * out_ap.ap[1][1] == round_up_to_multiple(num_idxs, 128)
    assert in_ap.ap[0][0] == elem_step
    stride_bytes_256 = exact_div(elem_step * mybir.dt.size(in_ap.dtype), 256)
    assert stride_bytes_256 < 256
    return gp.add_instruction(
        mybir.InstDMAGatherAnt(
            name=bassmod.get_next_instruction_name(),
            ins=[*gp.lower_ap_dma(in_ap, for_custom_bir_dma=True),
                 gp.lower_ap(idxs_ap),
                 gp.lower_val_access(gp.to_reg(num_idxs))],
            outs=[gp.lower_ap(out_ap)],
            transpose=False,
            num_idxs=num_idxs,
            elem_size=elem_size,
            stride_bytes_256=stride_bytes_256,
            gen_mode=0,
            single_packet=single_packet,
            queue_num=queue_num,
            sbuf_tokens_per_rank=0,
            sbuf_free_dim_per_rank=0,
            sbuf_free_dim_pad_per_rank=0,
            sbuf_byte_offset=0,
        ))


def build_bass(CA, CB):
    import concourse.bacc as bacc
    import concourse.bass as bassm
    import concourse.mybir as mybir
    import concourse.tile as tile
    from concourse.library_config import mlp
    from concourse.masks import make_identity

    f32 = mybir.dt.float32
    bf = mybir.dt.bfloat16 if USE_BF16 else f32
    i16 = mybir.dt.int16
    tbw = TBW if USE_BF16 else HID
    NCH = CA + CB
    capA, capB = CA * P, CB * P
    wA = GT * capA // 16
    wB = GT * capB // 16

    nc = bacc.Bacc("TRN2", target_bir_lowering=False)
    xT_d = nc.dram_tensor("xT", [P, SPC], bf, kind="ExternalInput")
    idxA_d = nc.dram_tensor("idxA", [P, TILES * capA // 16], i16, kind="ExternalInput")
    idxB_d = nc.dram_tensor("idxB", [P, TILES * capB // 16], i16, kind="ExternalInput")
    dest_d = nc.dram_tensor("dest", [P, TILES * NCH], bf, kind="ExternalInput")
    dinv_d = nc.dram_tensor("dinv", [P, TILES], f32, kind="ExternalInput")
    w1_d = nc.dram_tensor("w1", [IN_CH, HID], bf, kind="ExternalInput")
    w2_d = nc.dram_tensor("w2", [HID, HID2], f32, kind="ExternalInput")
    t1_d = nc.dram_tensor("t1", [P, HID], f32, kind="ExternalInput")
    t2_d = nc.dram_tensor("t2", [P, HID2], f32, kind="ExternalInput")
    fcw_d = nc.dram_tensor("fcw", [P, HID2], f32, kind="ExternalInput")
    y_d = nc.dram_tensor("y", [P, TILES], f32, kind="ExternalOutput")

    with tile.TileContext(nc) as tc:
        with (
            tc.tile_pool(name="const", bufs=1) as cpool,
            tc.tile_pool(name="upart", bufs=1) as upool,
            tc.tile_pool(name="ga", bufs=3) as gapool,
            tc.tile_pool(name="gb", bufs=2) as gbpool,
            tc.tile_pool(name="sel", bufs=20) as selpool,
            tc.tile_pool(name="work", bufs=4) as wpool,
            tc.tile_pool(name="pmm", bufs=2, space="PSUM") as pmm,
            tc.tile_pool(name="pacc", bufs=3, space="PSUM") as pacc,
            tc.tile_pool(name="ptr", bufs=1, space="PSUM") as ptr,
            tc.tile_pool(name="p3", bufs=2, space="PSUM") as p3pool,
            tc.tile_pool(name="dram", bufs=1, space="DRAM") as dpool,
        ):
            nc.gpsimd.load_library(mlp)

            # ---- constants ----
            idxA_t = cpool.tile([P, TILES * capA // 16], i16)
            nc.sync.dma_start(out=idxA_t[:], in_=idxA_d[:])
            idxB_t = cpool.tile([P, TILES * capB // 16], i16)
            nc.sync.dma_start(out=idxB_t[:], in_=idxB_d[:])
            dest_t = cpool.tile([P, TILES * NCH], bf)
            nc.sync.dma_start(out=dest_t[:], in_=dest_d[:])
            dinv_t = cpool.tile([P, TILES], f32)
            nc.sync.dma_start(out=dinv_t[:], in_=dinv_d[:])
            w1_t = cpool.tile([IN_CH, HID], bf)
            nc.sync.dma_start(out=w1_t[:], in_=w1_d[:])
            w2_t = cpool.tile([HID, HID2], f32)
            nc.sync.dma_start(out=w2_t[:], in_=w2_d[:])
            t1_t = cpool.tile([P, HID], f32)
            nc.sync.dma_start(out=t1_t[:], in_=t1_d[:])
            t2_t = cpool.tile([P, HID2], f32)
            nc.sync.dma_start(out=t2_t[:], in_=t2_d[:])
            fcw_t = cpool.tile([P, HID2], f32)
            nc.sync.dma_start(out=fcw_t[:], in_=fcw_d[:])

            ident = cpool.tile([P, P], f32)
            make_identity(nc, ident[:])
            ones_row = cpool.tile([1, P], f32)
            nc.gpsimd.memset(ones_row[:], 1.0)
            iota_i = cpool.tile([P, NCH * P], mybir.dt.int32)
            nc.gpsimd.iota(iota_i[:], pattern=[[0, NCH], [1, P]], base=0,
                           channel_multiplier=0)
            iota_b = cpool.tile([P, NCH * P], bf)
            nc.vector.tensor_copy(out=iota_b[:], in_=iota_i[:])

            u1_t = upool.tile([P, TILES * HID], f32, tag="u1")
            s2_t = upool.tile([P, TILES * HID], f32, tag="s2")
            out_t = upool.tile([P, TILES], f32, tag="out")

            ag1_in = dpool.tile([SPC, tbw], bf)
            s1_tab = dpool.tile([NSLOT, tbw], bf, addr_space="Shared")
            ag2_in = dpool.tile([SPC, tbw], bf)
            s2_tab = dpool.tile([NSLOT, tbw], bf, addr_space="Shared")

            # ---- L1 dense: u = (x*dinv) @ W1' ----
            xfull = cpool.tile([P, SPC], bf)
            nc.sync.dma_start(out=xfull[:], in_=xT_d[:])
            for t in range(TILES):
                pm = pmm.tile([P, HID], f32, space="PSUM", tag="pm")
                nc.tensor.matmul(out=pm[:], lhsT=xfull[:, t * P:(t + 1) * P],
                                 rhs=w1_t[:], start=True, stop=True)
                nc.scalar.activation(out=u1_t[:, t * HID:(t + 1) * HID], in_=pm[:],
                                     func=mybir.ActivationFunctionType.Copy)

            nc.gpsimd.dma_start(
                out=ag1_in[:].rearrange("(t p) w -> p t w", p=P)[:, :, 0:HID],
                in_=u1_t[:].rearrange("p (t f) -> p t f", f=HID),
            )
            nc.gpsimd.collective_compute(
                "AllGather", mybir.AluOpType.bypass,
                replica_groups=[list(range(NCORES))],
                ins=[ag1_in[:]], outs=[s1_tab[:]],
            )

            def tab_ap(tab, lo, cnt):
                return bassm.AP(tensor=tab[:].tensor, offset=lo * tbw,
                                ap=[[tbw, cnt], [1, HID]])

            def scatter_tiles(tab, u_tab, post):
                for g in range(NCALLS):
                    ga = gapool.tile([P, GT * CA, HID], bf, tag="ga")
                    _dma_gather_raw(
                        nc.gpsimd, nc, ga[:], tab_ap(tab, 0, LO_LIM),
                        idxA_t[:, g * wA:(g + 1) * wA], GT * capA, HID, tbw,
                        single_packet=False)
                    gb = gbpool.tile([P, GT * CB, HID], bf, tag="gb")
                    _dma_gather_raw(
                        nc.gpsimd, nc, gb[:], tab_ap(tab, HI_BASE, LO_LIM),
                        idxB_t[:, g * wB:(g + 1) * wB], GT * capB, HID, tbw,
                        single_packet=False)
                    for k in range(GT):
                        t = g * GT + k
                        sel = selpool.tile([P, NCH, P], bf, tag="sel")
                        nc.vector.tensor_tensor(
                            out=sel[:],
                            in0=dest_t[:, t * NCH:(t + 1) * NCH, None]
                                .to_broadcast([P, NCH, P]),
                            in1=iota_b[:].rearrange("p (c q) -> p c q", q=P),
                            op=mybir.AluOpType.is_equal,
                        )
                        acc = pacc.tile([P, HID], f32, space="PSUM", tag="acc")
                        for cc in range(NCH):
                            rhs = (ga[:, k * CA + cc, :] if cc < CA
                                   else gb[:, k * CB + cc - CA, :])
                            nc.tensor.matmul(out=acc[:], lhsT=sel[:, cc, :], rhs=rhs,
                                             start=(cc == 0), stop=False)
                        nc.tensor.matmul(out=acc[:], lhsT=ident[:],
                                         rhs=u_tab[:, t * HID:(t + 1) * HID],
                                         start=False, stop=True)
                        post(t, acc)

            # ---- L1 scatter + post: s2 = dinv * relu(dinv*(acc+u) + T1) ----
            def post1(t, acc):
                tmp2 = wpool.tile([P, HID], f32, tag="tmp2")
                nc.scalar.activation(out=tmp2[:], in_=acc[:],
                                     func=mybir.ActivationFunctionType.Copy,
                                     scale=dinv_t[:, t:t + 1])
                h1 = wpool.tile([P, HID], f32, tag="h1")
                nc.vector.tensor_tensor(out=h1[:], in0=tmp2[:], in1=t1_t[:],
                                        op=mybir.AluOpType.add)
                h1r = wpool.tile([P, HID], f32, tag="h1r")
                nc.scalar.activation(out=h1r[:], in_=h1[:],
                                     func=mybir.ActivationFunctionType.Relu)
                nc.scalar.activation(out=s2_t[:, t * HID:(t + 1) * HID],
                                     in_=h1r[:],
                                     func=mybir.ActivationFunctionType.Copy,
                                     scale=dinv_t[:, t:t + 1])

            scatter_tiles(s1_tab, u1_t, post1)

            nc.gpsimd.dma_start(
                out=ag2_in[:].rearrange("(t p) w -> p t w", p=P)[:, :, 0:HID],
                in_=s2_t[:].rearrange("p (t f) -> p t f", f=HID),
            )
            nc.gpsimd.collective_compute(
                "AllGather", mybir.AluOpType.bypass,
                replica_groups=[list(range(NCORES))],
                ins=[ag2_in[:]], outs=[s2_tab[:]],
            )

            # ---- L2 scatter + post ----
            def post2(t, acc):
                agg = wpool.tile([P, HID], f32, tag="agg")
                nc.scalar.activation(out=agg[:], in_=acc[:],
                                     func=mybir.ActivationFunctionType.Copy,
                                     scale=dinv_t[:, t:t + 1])
                trp = ptr.tile([HID, P], f32, space="PSUM", tag="trp")
                nc.tensor.transpose(out=trp[:], in_=agg[:], identity=ident[:])
                aggT = wpool.tile([HID, P], f32, tag="aggT")
                nc.scalar.activation(out=aggT[:], in_=trp[:],
                                     func=mybir.ActivationFunctionType.Copy)
                p3 = p3pool.tile([P, HID2], f32, space="PSUM", tag="p3")
                nc.tensor.matmul(out=p3[:], lhsT=aggT[:], rhs=w2_t[:],
                                 start=True, stop=False)
                nc.tensor.matmul(out=p3[:], lhsT=ones_row[:], rhs=t2_t[0:1, :],
                                 start=False, stop=True)
                h2 = wpool.tile([P, HID2], f32, tag="h2")
                nc.scalar.activation(out=h2[:], in_=p3[:],
                                     func=mybir.ActivationFunctionType.Relu)
                prod = wpool.tile([P, HID2], f32, tag="prod")
                nc.vector.tensor_tensor(out=prod[:], in0=h2[:], in1=fcw_t[:],
                                        op=mybir.AluOpType.mult)
                nc.vector.reduce_sum(out=out_t[:, t:t + 1], in_=prod[:],
                                     axis=mybir.AxisListType.X)

            scatter_tiles(s2_tab, s2_t, post2)

            nc.sync.dma_start(out=y_d[:], in_=out_t[:])

    nc.compile()
    return nc


# ----------------------------------------------------------------------
# entry point
# ----------------------------------------------------------------------
def prepare(inputs):
    inputs = {k: np.asarray(v) for k, v in inputs.items()}
    cores, consts = host_prep(**inputs)
    nc = build_bass(consts["CA"], consts["CB"])

    cast = BF16 if USE_BF16 else np.float32
    w2 = consts["W2p"].astype(np.float32)
    t1 = np.tile(consts["T1"][None, :], (P, 1)).astype(np.float32)
    t2 = np.tile(consts["T2"][None, :], (P, 1)).astype(np.float32)
    fcw = np.tile(consts["fcW"].reshape(1, -1), (P, 1)).astype(np.float32)

    in_maps = []
    for c in range(NCORES):
        in_maps.append({
            "xT": cores[c]["xT"].astype(BF16) if USE_BF16 else cores[c]["xT"],
            "idxA": cores[c]["idxA"],
            "idxB": cores[c]["idxB"],
            "dest": cores[c]["dest"].astype(cast),
            "dinv": cores[c]["dinv"],
            "w1": consts["W1p"].astype(BF16) if USE_BF16 else consts["W1p"],
            "w2": w2,
            "t1": t1,
            "t2": t2,
            "fcw": fcw,
        })
    return nc, in_maps, consts


def execute(nc, in_maps):
    from concourse.bass_utils import run_bass_kernel_spmd
    return run_bass_kernel_spmd(nc, in_maps, core_ids=list(range(NCORES)))


def unshard(res, consts):
    y = np.zeros((N_NODES, 1), np.float32)
    nos = consts["node_of_slot"]
    fcb = consts["fcb"]
    for c in range(NCORES):
        nodes = nos[c * SPC:(c + 1) * SPC]
        occ = nodes >= 0
        vals = res.results[c]["y"].T.reshape(-1) + fcb
        y[nodes[occ], 0] = vals[occ]
    return y


def kernel(**inputs):
    nc, in_maps, consts = prepare(inputs)
    res = execute(nc, in_maps)
    return unshard(res, consts)



# revision 19
# speedup vs baseline: 1.8324x; 1.8324x over previous
"""Distributed 2-layer GCN (BangaloreGCN) on 8 Trainium2 NeuronCores.

Source-partitioned strategy (node/graph parallel per the sharding hint,
with the cross-core reduction done by ReduceScatter instead of
AllGather):

  * Nodes are packed into 424 global dest tiles x 128 lanes; lanes
    [16c, 16c+16) of every tile belong to core c, so each core owns
    6784 node slots.  A color-aware greedy pack balances, for every
    (src core, dest tile) pair, the number of incoming edges to
    <= 256, so every dest tile needs exactly ceil/128 = NCH_b chunks
    (identical across cores -> one static SPMD program).
  * GCN algebra: per layer the table s = dinv*h is computed locally
    (8x less dense work), each core gathers s[src] for the edges whose
    SOURCE it owns from its local table, and scatter-adds them into a
    full-size partial accumulator [128 lanes, 424*64] with one-hot
    selection matmuls in PSUM.  A bf16 ReduceScatter then hands every
    core the complete sums for its own 16-lane slab.
  * One-hot masks are built per chunk with tensor_scalar(is_equal)
    (iota vs the chunk's dest-lane column), which runs in the DVE 4x
    perf mode.
  * Layer 2 applies W2 *before* aggregation (A@(hW2) == (A@h)W2), so
    its table rows are 32 wide: half the gather bytes and half the
    collective payload of layer 1.
"""

import sys

sys.path.insert(0, "/opt/trn_rl_repo")

import heapq

import ml_dtypes
import numpy as np

BF16 = ml_dtypes.bfloat16

# ---- problem constants (hardcoded per contest contract) ----
N_NODES = 50000
IN_CH = 128
HID = 64
HID2 = 32
BN_EPS = 1e-5

NCORES = 8
P = 128
T_ALL = 424                # global dest tiles
SLAB = 16                  # lanes per core in each tile
LOCN = T_ALL * SLAB        # local node slots per core (6784)
LT = LOCN // P             # local col-tiles (53)
NG = 64                    # chunks per dma_gather call
PAD_LANE = 200.0
TBW = 128                  # table row stride in bf16 elems (256B)
GRP1 = 8                   # dest bins per PSUM bank, layer 1 (8*64 = 512 f32)
GRP2 = 16                  # dest bins per PSUM bank, layer 2 (16*32 = 512 f32)


# ----------------------------------------------------------------------
# host-side preparation
# ----------------------------------------------------------------------
def _pack_nodes(row, col, n):
    """Assign every node an owner slab and a global dest tile + lane.

    Returns (lane, tile) per node: owner core = lane // 16."""
    deg_in = np.bincount(col, minlength=n)
    outdeg = np.bincount(row, minlength=n)

    # owner slabs: balance out-degree (work per core), capacity LOCN
    order = np.argsort(-outdeg, kind="stable")
    heap = [(0, c) for c in range(NCORES)]
    heapq.heapify(heap)
    cnt = np.zeros(NCORES, np.int64)
    owner = np.empty(n, np.int8)
    for v in order:
        load, c = heapq.heappop(heap)
        owner[v] = c
        cnt[c] += 1
        if cnt[c] < LOCN:
            heapq.heappush(heap, (load + int(outdeg[v]), c))

    # per-node in-edge color vector (color = owner of the source node)
    cc = np.zeros((n, NCORES), np.int32)
    np.add.at(cc, (col, owner[row].astype(np.int64)), 1)

    # color-aware greedy tile packing: keep max_c E_cb small
    loads = np.zeros((T_ALL, NCORES), np.int64)
    cap = np.full((T_ALL, NCORES), SLAB, np.int16)
    tile_of = np.empty(n, np.int32)
    lane_of = np.empty(n, np.int32)
    BIG = 1 << 40
    for v in np.argsort(-deg_in, kind="stable"):
        c = int(owner[v])
        score = (loads + cc[v][None, :]).max(axis=1)
        score[cap[:, c] <= 0] = BIG
        t = int(np.argmin(score))
        tile_of[v] = t
        lane_of[v] = SLAB * c + (SLAB - cap[t, c])
        loads[t] += cc[v]
        cap[t, c] -= 1
    return lane_of, tile_of, loads


def _wrap_idx(arr):
    ni = arr.shape[0]
    blk = arr.reshape(ni // 16, 16).T.astype(np.int16)
    return np.tile(blk, (8, 1))


def host_prep(x, edge_index, W1, b1, W2, b2, fcW, fcb,
              g1, be1, rm1, rv1, g2, be2, rm2, rv2):
    n = x.shape[0]
    row = np.asarray(edge_index[0], np.int64)
    col = np.asarray(edge_index[1], np.int64)

    deg = np.bincount(col, minlength=n).astype(np.float32) + 1.0
    dinv = (1.0 / np.sqrt(deg)).astype(np.float32)

    lane, tile = _pack_nodes(row, col, n)[:2]
    owner = lane // SLAB
    a = lane % SLAB
    part = a * 8 + (tile % 8)          # SBUF partition in compute layout
    ct = tile // 8                     # SBUF col-tile in compute layout
    q = ct * P + part                  # compute index within core
    # gather tables are stored in compute order: table row == q

    # ---- per-(core, bin) chunk schedule, identical across cores ----
    e_core = owner[row]
    e_src = q[row].astype(np.int16)
    e_lane = lane[col].astype(np.float32)
    e_bin = tile[col]
    cnt_cb = np.zeros((NCORES, T_ALL), np.int64)
    np.add.at(cnt_cb, (e_core, e_bin), 1)
    ncb = np.maximum(1, -(-cnt_cb.max(axis=0) // P)).astype(np.int64)
    ncht = int(ncb.sum())
    bin_chunk_off = np.concatenate([[0], np.cumsum(ncb)])  # chunk offsets
    slot_off = bin_chunk_off * P

    # ---- per-core edge streams ----
    cores = []
    for c in range(NCORES):
        m = e_core == c
        sbin = e_bin[m]
        order = np.argsort(sbin, kind="stable")
        sbin = sbin[order]
        ssrc = e_src[m][order]
        slane = e_lane[m][order]
        starts = np.searchsorted(sbin, np.arange(T_ALL))
        rank = np.arange(len(sbin)) - starts[sbin]
        pos = slot_off[sbin] + rank
        idx_stream = np.zeros(ncht * P, np.int16)
        lane_stream = np.full(ncht * P, PAD_LANE, np.float32)
        idx_stream[pos] = ssrc
        lane_stream[pos] = slane

        calls = []
        k = 0
        while k < ncht:
            L = min(NG, ncht - k)
            calls.append(_wrap_idx(idx_stream[k * P:(k + L) * P]))
            k += L
        idx_img = np.hstack(calls)
        dest_img = lane_stream.reshape(ncht, P).T.copy()

        nodes_c = np.where(owner == c)[0]
        xs = np.zeros((LOCN, IN_CH), np.float32)
        xs[q[nodes_c]] = x[nodes_c] * dinv[nodes_c, None]
        dv = np.zeros(LOCN, np.float32)
        dv[q[nodes_c]] = dinv[nodes_c]
        dvpt = dv.reshape(LT, P).T                      # [128, LT]
        cores.append(dict(
            idx=idx_img, dest=dest_img,
            xT=np.ascontiguousarray(xs.T).astype(BF16),
            dinvimg=np.repeat(dvpt, HID, axis=1).astype(BF16),
            d2img=np.repeat(dvpt, HID2, axis=1).astype(BF16),
        ))

    S1c = (g1 / np.sqrt(rv1 + BN_EPS)).astype(np.float32)
    T1 = ((b1 - rm1) * S1c + be1).astype(np.float32)
    S2c = (g2 / np.sqrt(rv2 + BN_EPS)).astype(np.float32)
    T2 = ((b2 - rm2) * S2c + be2).astype(np.float32)
    consts = dict(
        w1=(W1 * S1c[None, :]).astype(BF16),
        w2=(W2 * S2c[None, :]).astype(BF16),
        t1=np.tile(T1[None, :], (P, 1)).astype(BF16),
        t2=np.tile(T2[None, :], (P, 1)).astype(BF16),
        fcw=np.tile(np.asarray(fcW, np.float32).reshape(1, -1), (P, 1)).astype(BF16),
        fcb=float(np.asarray(fcb).reshape(-1)[0]),
        ncb=ncb, ncht=ncht,
        owner=owner, part=part, ct=ct,
    )
    return cores, consts


# ----------------------------------------------------------------------
# device program
# ----------------------------------------------------------------------
def _dma_gather_raw(gp, bassmod, out_ap, in_ap, idxs_ap, num_idxs, elem_size,
                    elem_step, single_packet=False, queue_num=0):
    """bass.dma_gather allowing elem_size_bytes below 256B (row stride must
    still be a multiple of 256B)."""
    import concourse.mybir as mybir
    from concourse import ap_utils
    from concourse.bass import MemorySpace, exact_div, round_up_to_multiple

    assert idxs_ap.dtype == mybir.dt.int16
    assert in_ap.dtype == out_ap.dtype
    assert in_ap.space == MemorySpace.DRAM
    assert idxs_ap.space == MemorySpace.SBUF and out_ap.space == MemorySpace.SBUF
    assert ap_utils.ap_is_contiguous(out_ap.ap[1:])
    assert ap_utils.ap_is_contiguous(idxs_ap.ap[1:])
    assert in_ap.ap[-1][1] == out_ap.ap[-1][1] == elem_size
    assert out_ap.ap[0][1] * out_ap.ap[1][1] == round_up_to_multiple(num_idxs, 128)
    assert in_ap.ap[0][0] == elem_step
    stride_bytes_256 = exact_div(elem_step * mybir.dt.size(in_ap.dtype), 256)
    assert stride_bytes_256 < 256
    return gp.add_instruction(
        mybir.InstDMAGatherAnt(
            name=bassmod.get_next_instruction_name(),
            ins=[*gp.lower_ap_dma(in_ap, for_custom_bir_dma=True),
                 gp.lower_ap(idxs_ap),
                 gp.lower_val_access(gp.to_reg(num_idxs))],
            outs=[gp.lower_ap(out_ap)],
            transpose=False,
            num_idxs=num_idxs,
            elem_size=elem_size,
            stride_bytes_256=stride_bytes_256,
            gen_mode=0,
            single_packet=single_packet,
            queue_num=queue_num,
            sbuf_tokens_per_rank=0,
            sbuf_free_dim_per_rank=0,
            sbuf_free_dim_pad_per_rank=0,
            sbuf_byte_offset=0,
        ))


def build_bass(ncb, ncht):
    import concourse.bacc as bacc
    import concourse.bass as bassm
    import concourse.mybir as mybir
    import concourse.tile as tile
    from concourse.library_config import mlp
    from concourse.masks import make_identity

    f32 = mybir.dt.float32
    bf = mybir.dt.bfloat16
    i16 = mybir.dt.int16
    Act = mybir.ActivationFunctionType
    Alu = mybir.AluOpType

    bin_chunk_off = np.concatenate([[0], np.cumsum(ncb)])

    nc = bacc.Bacc("TRN2", target_bir_lowering=False)
    xT_d = nc.dram_tensor("xT", [P, LOCN], bf, kind="ExternalInput")
    idx_d = nc.dram_tensor("idx", [P, ncht * 8], i16, kind="ExternalInput")
    dest_d = nc.dram_tensor("dest", [P, ncht], f32, kind="ExternalInput")
    dinvimg_d = nc.dram_tensor("dinvimg", [P, LT * HID], bf, kind="ExternalInput")
    d2img_d = nc.dram_tensor("d2img", [P, LT * HID2], bf, kind="ExternalInput")
    w1_d = nc.dram_tensor("w1", [IN_CH, HID], bf, kind="ExternalInput")
    w2_d = nc.dram_tensor("w2", [HID, HID2], bf, kind="ExternalInput")
    t1_d = nc.dram_tensor("t1", [P, HID], bf, kind="ExternalInput")
    t2_d = nc.dram_tensor("t2", [P, HID2], bf, kind="ExternalInput")
    fcw_d = nc.dram_tensor("fcw", [P, HID2], bf, kind="ExternalInput")
    y_d = nc.dram_tensor("y", [P, LT], f32, kind="ExternalOutput")

    with tile.TileContext(nc) as tc:
        with (
            tc.tile_pool(name="const", bufs=1) as cpool,
            tc.tile_pool(name="upart", bufs=1) as upool,
            tc.tile_pool(name="ga", bufs=3) as gapool,
            tc.tile_pool(name="sel", bufs=12) as selpool,
            tc.tile_pool(name="asb", bufs=3) as asbpool,
            tc.tile_pool(name="hts", bufs=2) as htspool,
            tc.tile_pool(name="pacc", bufs=3, space="PSUM") as pacc,
            tc.tile_pool(name="ps2", bufs=2, space="PSUM") as ps2p,
            tc.tile_pool(name="ptp", bufs=2, space="PSUM") as ptpp,
            tc.tile_pool(name="dram", bufs=1, space="DRAM") as dpool,
        ):
            nc.gpsimd.load_library(mlp)

            # ---- constants ----
            xfull = cpool.tile([P, LOCN], bf)
            nc.sync.dma_start(out=xfull[:], in_=xT_d[:])
            w1_t = cpool.tile([IN_CH, HID], bf)
            nc.sync.dma_start(out=w1_t[:], in_=w1_d[:])
            idx_t = cpool.tile([P, ncht * 8], i16)
            nc.sync.dma_start(out=idx_t[:], in_=idx_d[:])
            dest_t = cpool.tile([P, ncht], f32)
            nc.sync.dma_start(out=dest_t[:], in_=dest_d[:])
            dinvimg = cpool.tile([P, LT * HID], bf)
            nc.sync.dma_start(out=dinvimg[:], in_=dinvimg_d[:])
            d2img = cpool.tile([P, LT * HID2], bf)
            nc.sync.dma_start(out=d2img[:], in_=d2img_d[:])
            w2_t = cpool.tile([HID, HID2], bf)
            nc.sync.dma_start(out=w2_t[:], in_=w2_d[:])
            t1_t = cpool.tile([P, HID], bf)
            nc.sync.dma_start(out=t1_t[:], in_=t1_d[:])
            t2_t = cpool.tile([P, HID2], bf)
            nc.sync.dma_start(out=t2_t[:], in_=t2_d[:])
            fcw_t = cpool.tile([P, HID2], bf)
            nc.sync.dma_start(out=fcw_t[:], in_=fcw_d[:])

            iota_i = cpool.tile([P, P], mybir.dt.int32)
            nc.gpsimd.iota(iota_i[:], pattern=[[1, P]], base=0,
                           channel_multiplier=0)
            iota_b = cpool.tile([P, P], bf)
            nc.vector.tensor_copy(out=iota_b[:], in_=iota_i[:])
            ident_b = cpool.tile([P, P], bf)
            make_identity(nc, ident_b[:])

            u1bf = upool.tile([P, LT, HID], bf, tag="u1")
            s2bf = upool.tile([P, LT, HID2], bf, tag="s2")
            agg1 = upool.tile([P, LT, HID], bf, tag="agg1")
            agg2 = upool.tile([P, LT, HID2], bf, tag="agg2")
            h1 = upool.tile([P, LT, HID], bf, tag="h1")
            h2 = upool.tile([P, LT, HID2], bf, tag="h2")
            s2raw = upool.tile([P, LT * HID2], bf, tag="s2raw")
            y_sb = upool.tile([P, LT], f32, tag="y")
            scr = upool.tile([P, HID2], bf, tag="scr")

            tab1 = dpool.tile([LOCN, TBW], bf)
            tab2 = dpool.tile([LOCN, TBW], bf)
            part1 = dpool.tile([P, T_ALL * HID], bf)
            part2 = dpool.tile([P, T_ALL * HID2], bf)
            rs1 = dpool.tile([SLAB, T_ALL * HID], bf)
            rs2 = dpool.tile([SLAB, T_ALL * HID2], bf)

            # ---- L1 dense: u1 = (dinv*x) @ W1' ----
            for g in range(0, LT, 8):
                gl = min(8, LT - g)
                pm = ps2p.tile([P, GRP2, HID2], f32, space="PSUM", tag="ps2")
                pmv = pm[:].rearrange("p a w -> p (a w)")
                for j in range(gl):
                    nc.tensor.matmul(out=pmv[:, j * HID:(j + 1) * HID],
                                     lhsT=xfull[:, (g + j) * P:(g + j + 1) * P],
                                     rhs=w1_t[:], start=True, stop=True)
                nc.scalar.activation(
                    out=u1bf[:, g:g + gl, :].rearrange("p a w -> p (a w)"),
                    in_=pmv[:, 0:gl * HID], func=Act.Copy)
            nc.sync.dma_start(
                out=bassm.AP(tensor=tab1[:].tensor, offset=0,
                             ap=[[TBW, P], [TBW * P, LT], [1, HID]]),
                in_=u1bf[:])

            # ---- scatter: gather + one-hot matmul accumulate + drain ----
            def scatter(tab, width, partial, grp):
                tab_ap = bassm.AP(tensor=tab[:].tensor, offset=0,
                                  ap=[[TBW, LOCN], [1, width]])
                ngrp = -(-T_ALL // grp)
                acc = None
                accv = None
                call_start = 0
                call_len = 0
                ga = None
                for b in range(T_ALL):
                    gi, sl = divmod(b, grp)
                    gl = min(grp, T_ALL - gi * grp)
                    if sl == 0:
                        acc = pacc.tile([P, grp, width] if width == HID
                                        else [P, grp, width],
                                        f32, space="PSUM", tag="acc")
                        accv = acc[:].rearrange("p a w -> p (a w)")
                    for j in range(int(ncb[b])):
                        k = int(bin_chunk_off[b]) + j
                        if k == call_start + call_len:
                            call_start = k
                            call_len = min(NG, ncht - k)
                            ga = gapool.tile([P, call_len, width], bf, tag="ga")
                            _dma_gather_raw(
                                nc.gpsimd, nc, ga[:], tab_ap,
                                idx_t[:, call_start * 8:
                                      (call_start + call_len) * 8],
                                call_len * P, width, TBW)
                        sel = selpool.tile([P, P], bf, tag="sel")
                        nc.vector.tensor_scalar(
                            out=sel[:], in0=iota_b[:],
                            scalar1=dest_t[:, k:k + 1], scalar2=None,
                            op0=Alu.is_equal)
                        nc.tensor.matmul(
                            out=accv[:, sl * width:(sl + 1) * width],
                            lhsT=sel[:], rhs=ga[:, k - call_start, :],
                            start=(j == 0), stop=(j == int(ncb[b]) - 1))
                    if sl == gl - 1:
                        asb = asbpool.tile([P, grp * width], bf, tag="asb")
                        nc.scalar.activation(out=asb[:, 0:gl * width],
                                             in_=accv[:, 0:gl * width],
                                             func=Act.Copy)
                        off = gi * grp * width
                        nc.sync.dma_start(
                            out=partial[:, off:off + gl * width],
                            in_=asb[:, 0:gl * width])

            scatter(tab1, HID, part1, GRP1)

            nc.gpsimd.collective_compute(
                "ReduceScatter", mybir.AluOpType.add,
                replica_groups=[list(range(NCORES))],
                ins=[part1[:]], outs=[rs1[:]],
            )

            # ---- post1: h1 = relu(dinv*(agg+u1) + T1); s2 = dinv*(h1@W2') ----
            for a in range(SLAB):
                nc.sync.dma_start(
                    out=agg1[a * 8:(a + 1) * 8, :, :],
                    in_=bassm.AP(tensor=rs1[:].tensor,
                                 offset=a * T_ALL * HID,
                                 ap=[[HID, 8], [8 * HID, LT], [1, HID]]))
            a1v = agg1[:].rearrange("p c w -> p (c w)")
            u1v = u1bf[:].rearrange("p c w -> p (c w)")
            h1v = h1[:].rearrange("p c w -> p (c w)")
            nc.vector.tensor_tensor(out=a1v[:], in0=a1v[:], in1=u1v[:],
                                    op=Alu.add)
            nc.vector.tensor_tensor(out=a1v[:], in0=a1v[:], in1=dinvimg[:],
                                    op=Alu.mult)
            nc.vector.tensor_tensor(
                out=agg1[:], in0=agg1[:],
                in1=t1_t[:, None, :].to_broadcast([P, LT, HID]),
                op=Alu.add)
            nc.scalar.activation(out=h1v[:], in_=a1v[:], func=Act.Relu)

            # transpose h1 per 8 col-tiles, then apply W2
            for g in range(0, LT, 8):
                gl = min(8, LT - g)
                tp = ptpp.tile([HID, 8, P], bf, space="PSUM", tag="tp")
                for j in range(gl):
                    nc.tensor.transpose(out=tp[:, j, :],
                                        in_=h1[:, g + j, :],
                                        identity=ident_b[:])
                hts = htspool.tile([HID, 8 * P], bf, tag="hts")
                nc.scalar.activation(
                    out=hts[:, 0:gl * P],
                    in_=tp[:].rearrange("p a w -> p (a w)")[:, 0:gl * P],
                    func=Act.Copy)
                gi2, r2 = divmod(g // 8, 2)
                if r2 == 0:
                    pm2 = ps2p.tile([P, GRP2, HID2], f32, space="PSUM",
                                    tag="ps2")
                    pm2v = pm2[:].rearrange("p a w -> p (a w)")
                for j in range(gl):
                    nc.tensor.matmul(
                        out=pm2v[:, (r2 * 8 + j) * HID2:
                                 (r2 * 8 + j + 1) * HID2],
                        lhsT=hts[:, j * P:(j + 1) * P], rhs=w2_t[:],
                        start=True, stop=True)
                if r2 == 1 or g + 8 >= LT:
                    lo = gi2 * GRP2 * HID2
                    ln = (r2 * 8 + gl) * HID2
                    nc.scalar.activation(out=s2raw[:, lo:lo + ln],
                                         in_=pm2v[:, 0:ln], func=Act.Copy)
            nc.vector.tensor_tensor(
                out=s2bf[:].rearrange("p c w -> p (c w)"), in0=s2raw[:],
                in1=d2img[:], op=Alu.mult)
            nc.sync.dma_start(
                out=bassm.AP(tensor=tab2[:].tensor, offset=0,
                             ap=[[TBW, P], [TBW * P, LT], [1, HID2]]),
                in_=s2bf[:])

            # ---- L2 scatter ----
            scatter(tab2, HID2, part2, GRP2)

            nc.gpsimd.collective_compute(
                "ReduceScatter", mybir.AluOpType.add,
                replica_groups=[list(range(NCORES))],
                ins=[part2[:]], outs=[rs2[:]],
            )

            # ---- post2 + readout ----
            for a in range(SLAB):
                nc.sync.dma_start(
                    out=agg2[a * 8:(a + 1) * 8, :, :],
                    in_=bassm.AP(tensor=rs2[:].tensor,
                                 offset=a * T_ALL * HID2,
                                 ap=[[HID2, 8], [8 * HID2, LT], [1, HID2]]))
            a2v = agg2[:].rearrange("p c w -> p (c w)")
            s2v = s2bf[:].rearrange("p c w -> p (c w)")
            h2v = h2[:].rearrange("p c w -> p (c w)")
            nc.vector.tensor_tensor(out=a2v[:], in0=a2v[:], in1=s2v[:],
                                    op=Alu.add)
            nc.vector.tensor_tensor(out=a2v[:], in0=a2v[:], in1=d2img[:],
                                    op=Alu.mult)
            nc.vector.tensor_tensor(
                out=agg2[:], in0=agg2[:],
                in1=t2_t[:, None, :].to_broadcast([P, LT, HID2]),
                op=Alu.add)
            nc.scalar.activation(out=h2v[:], in_=a2v[:], func=Act.Relu)
            nc.vector.tensor_tensor(
                out=h2[:], in0=h2[:],
                in1=fcw_t[:, None, :].to_broadcast([P, LT, HID2]),
                op=Alu.mult)
            for c in range(LT):
                nc.vector.reduce_sum(out=y_sb[:, c:c + 1], in_=h2[:, c, :],
                                     axis=mybir.AxisListType.X)
            nc.sync.dma_start(out=y_d[:], in_=y_sb[:])

    nc.compile()
    return nc


# ----------------------------------------------------------------------
# entry point
# ----------------------------------------------------------------------
def prepare(inputs):
    inputs = {k: np.asarray(v) for k, v in inputs.items()}
    cores, consts = host_prep(**inputs)
    nc = build_bass(consts["ncb"], consts["ncht"])

    in_maps = []
    for c in range(NCORES):
        in_maps.append({
            "xT": cores[c]["xT"],
            "idx": cores[c]["idx"],
            "dest": cores[c]["dest"],
            "dinvimg": cores[c]["dinvimg"],
            "d2img": cores[c]["d2img"],
            "w1": consts["w1"],
            "w2": consts["w2"],
            "t1": consts["t1"],
            "t2": consts["t2"],
            "fcw": consts["fcw"],
        })
    return nc, in_maps, consts


def execute(nc, in_maps):
    from concourse.bass_utils import run_bass_kernel_spmd
    return run_bass_kernel_spmd(nc, in_maps, core_ids=list(range(NCORES)))


def unshard(res, consts):
    y = np.zeros((N_NODES, 1), np.float32)
    owner, part, ct = consts["owner"], consts["part"], consts["ct"]
    fcb = consts["fcb"]
    pc = np.stack([np.asarray(res.results[c]["y"], np.float32)
                   for c in range(NCORES)])
    y[:, 0] = pc[owner[:N_NODES], part[:N_NODES], ct[:N_NODES]] + fcb
    return y


def kernel(**inputs):
    nc, in_maps, consts = prepare(inputs)
    res = execute(nc, in_maps)
    return unshard(res, consts)
